# revision 11
# baseline (speedup 1.0000x reference)
"""ChunkBySlices Trainium2 kernel.

For each batch row n: out[n, t] = x[n, start_n + t] for t < slice_len_n, else 0
(start/end from `slices`, clamped by `lens`). Pure data movement, so the device
kernel is DMA-only:

  - N=64 rows sharded across 8 cores (strided: core c takes rows c, c+8, ...).
  - Host precomputes, per 16-row output block, a gather index into a per-core
    source buffer (the core's x shard + small host-built "aux" blocks covering
    the valid/invalid boundary + a zero guard block). Invalid blocks get an
    out-of-bounds sentinel.
  - Device: memset SBUF tile -> indirect-DMA gather (128 x 16KB descriptors,
    OOB indices skipped via bounds_check) -> contiguous 2MB store per row.

HBM traffic per core ~= 16MB write + (valid bytes) read, which is the minimum
for this op.
"""

import numpy as np

N, T, F = 64, 2048, 256
NCORES = 8
N_LOC = N // NCORES          # 8 batch rows per core
BLK = 16                     # t-rows per gather descriptor (16KB)
NBLK = T // BLK              # 128 blocks per batch row = SBUF partitions
AUXB = 2                     # aux (boundary) blocks per batch row
SRC_ROWS = N_LOC * T + N_LOC * AUXB * BLK   # 16384 + 256 = 16640
SRC_ALLOC = SRC_ROWS + BLK                  # + zero block
SENTINEL = SRC_ROWS                         # invalid blocks gather the zero block

_CACHE = {}


BUFS = 8          # SBUF slots (16KB/partition each; 8 -> 128KB of 192KB)
BOUNDS = SRC_ROWS - BLK


def _build_bass(reps=1, variant="plain"):
    """Raw-bass pipelined kernel.

    variant="plain":  every block (incl. invalid) is gathered; invalid blocks
                      read the zero block appended to src. No compute engines.
    variant="memset": DVE pre-zeros each tile; the gather skips invalid
                      (sentinel) indices via bounds_check, saving the HBM
                      reads of zeros for the invalid tail.
    """
    import concourse.bass as bass
    import concourse.mybir as mybir

    nc = bass.Bass()
    src = nc.dram_tensor("src", [SRC_ALLOC, F], mybir.dt.float32,
                         kind="ExternalInput")
    idx = nc.dram_tensor("idx", [NBLK, N_LOC], mybir.dt.int32,
                         kind="ExternalInput")
    out = nc.dram_tensor("out", [N_LOC * NBLK, BLK * F], mybir.dt.float32,
                         kind="ExternalOutput")

    G = reps * N_LOC
    use_memset = variant == "memset"
    W = BLK * F

    from contextlib import ExitStack
    with ExitStack() as es:
        big = es.enter_context(
            nc.sbuf_tensor([NBLK, BUFS * W], mybir.dt.float32))
        idxsb = es.enter_context(
            nc.sbuf_tensor([NBLK, N_LOC], mybir.dt.int32))
        # One semaphore lane per pipeline slot; at most one outstanding DMA
        # per lane, and every inc is preceded by an engine-local wait on the
        # previous boundary (sem increments of concurrent DMAs are unordered,
        # so a shared counter cannot sequence a pipeline).
        sw = [es.enter_context(nc.semaphore(f"sw{j}")) for j in range(BUFS)]
        hw = [es.enter_context(nc.semaphore(f"hw{j}")) for j in range(BUFS)]
        hx = es.enter_context(nc.semaphore("hx"))   # idx load
        ve = es.enter_context(nc.semaphore("ve"))   # memset completions (+1)
        block = es.enter_context(nc.Block())

        def slot(g):
            s = (g % BUFS) * W
            return big[:, s:s + W]

        @block.sync
        def _(sync):
            sync.dma_start(out=idxsb[:], in_=idx[:]).then_inc(hx, 16)
            for g in range(G):
                j, k = g % BUFS, g // BUFS
                sync.wait_ge(sw[j], 16 * (k + 1))     # gather g complete
                if g >= BUFS:
                    sync.wait_ge(hw[j], 16 * k)       # lane free
                n = g % N_LOC
                sync.dma_start(out=out[n * NBLK:(n + 1) * NBLK, :],
                               in_=slot(g)).then_inc(hw[j], 16)
            for j in range(BUFS):
                sync.wait_ge(hw[j], 16 * (G // BUFS))

        if use_memset:
            @block.vector
            def _(vector):
                for g in range(G):
                    j, k = g % BUFS, g // BUFS
                    if g >= BUFS:
                        # slot reused: wait for store g-BUFS to complete
                        vector.wait_ge(hw[j], 16 * k)
                    vector.memset(slot(g), 0.0).then_inc(ve, 1)

        @block.gpsimd
        def _(gpsimd):
            gpsimd.wait_ge(hx, 16)          # idx loaded
            for g in range(G):
                j, k = g % BUFS, g // BUFS
                if use_memset:
                    gpsimd.wait_ge(ve, g + 1)
                    if g >= BUFS:
                        gpsimd.wait_ge(sw[j], 16 * k)   # lane free
                elif g >= BUFS:
                    gpsimd.wait_ge(hw[j], 16 * k)   # store g-BUFS complete
                    gpsimd.wait_ge(sw[j], 16 * k)   # lane free
                n = g % N_LOC
                kw = {}
                if use_memset:
                    kw = dict(bounds_check=BOUNDS, oob_is_err=False)
                gpsimd.indirect_dma_start(
                    out=slot(g),
                    out_offset=None,
                    in_=src[:],
                    in_offset=bass.IndirectOffsetOnAxis(
                        ap=idxsb[:, n:n + 1], axis=0),
                    **kw,
                ).then_inc(sw[j], 16)

    return nc


class _Runner:
    """Compile-once PJRT executor for a Bass module on NCORES cores.

    Mirrors concourse.bass2jax.run_bass_via_pjrt's multi-core path, but keeps
    the jitted executable so repeated kernel() calls don't retrace/recompile.
    """

    def __init__(self, nc):
        import jax
        import concourse.mybir as mybir
        from concourse import bass2jax
        from jax.experimental.shard_map import shard_map
        from jax.sharding import Mesh, PartitionSpec

        bass2jax.install_neuronx_cc_hook()
        assert nc.dbg_addr is None
        partition_name = (nc.partition_id_tensor.name
                          if nc.partition_id_tensor else None)

        in_names, out_names, out_avals = [], [], []
        for alloc in nc.m.functions[0].allocations:
            if not isinstance(alloc, mybir.MemoryLocationSet):
                continue
            name = alloc.memorylocations[0].name
            if alloc.kind == "ExternalInput":
                if name != partition_name:
                    in_names.append(name)
            elif alloc.kind == "ExternalOutput":
                out_names.append(name)
                out_avals.append(jax.core.ShapedArray(
                    tuple(alloc.tensor_shape), mybir.dt.np(alloc.dtype)))
        self.in_names = list(in_names)
        self.out_names = out_names
        self.out_avals = out_avals
        n_params = len(in_names)
        all_names = in_names + out_names
        if partition_name is not None:
            all_names = all_names + [partition_name]
        donate = tuple(range(n_params, n_params + len(out_names)))

        def _body(*args):
            operands = list(args)
            if partition_name is not None:
                operands.append(bass2jax.partition_id_tensor())
            return tuple(bass2jax._bass_exec_p.bind(
                *operands,
                out_avals=tuple(out_avals),
                in_names=tuple(all_names),
                out_names=tuple(out_names),
                lowering_input_output_aliases=(),
                sim_require_finite=True,
                sim_require_nnan=True,
                nc=nc,
            ))

        devices = jax.devices()[:NCORES]
        assert len(devices) == NCORES, f"need {NCORES} cores, have {devices}"
        mesh = Mesh(np.asarray(devices), ("core",))
        nin = n_params + len(out_names)
        self.sharded = jax.jit(
            shard_map(_body, mesh=mesh,
                      in_specs=(PartitionSpec("core"),) * nin,
                      out_specs=(PartitionSpec("core"),) * len(out_names),
                      check_rep=False),
            donate_argnums=donate, keep_unused=True)

    def __call__(self, in_maps):
        concat_in = [np.concatenate([m[name] for m in in_maps], axis=0)
                     for name in self.in_names]
        zeros = [np.zeros((NCORES * a.shape[0], *a.shape[1:]), a.dtype)
                 for a in self.out_avals]
        out_arrs = self.sharded(*concat_in, *zeros)
        return [
            {name: np.asarray(out_arrs[i]).reshape(
                NCORES, *self.out_avals[i].shape)[c]
             for i, name in enumerate(self.out_names)}
            for c in range(NCORES)
        ]


VARIANT = "plain"


def _get_runner(reps=1, variant=None):
    variant = VARIANT if variant is None else variant
    key = ("runner", reps, variant)
    if key not in _CACHE:
        _CACHE[key] = _Runner(_build_bass(reps, variant))
    return _CACHE[key]


def _prepare_in_maps(x, slices_in, lens_in):
    """Host-side index/aux computation. Returns (in_maps, chunk_lens)."""

    start = slices_in[:, 0].astype(np.int64)
    end = slices_in[:, 1].astype(np.int64)
    lens64 = lens_in.astype(np.int64)

    chunk_lens = np.maximum(end - start, 0)
    empty = chunk_lens == 0
    left_pad = np.where(empty, 0, np.maximum(-start, 0))
    start_ = np.maximum(start, 0)
    end_ = np.minimum(end, lens64)
    slice_lens = np.maximum(end_ - start_, 0)
    lo = left_pad                     # valid t interval [lo, hi)
    hi = left_pad + slice_lens
    src0 = start_ - left_pad          # src row for t=0

    t0 = np.arange(NBLK, dtype=np.int64) * BLK          # (NBLK,)
    t1 = t0 + BLK
    full = (t0[None, :] >= lo[:, None]) & (t1[None, :] <= hi[:, None])
    none = (t1[None, :] <= lo[:, None]) | (t0[None, :] >= hi[:, None])
    partial = ~(full | none)                            # (N, NBLK)

    # Host-built boundary blocks (exact reference math on <=2 blocks per row).
    aux = np.zeros((N, AUXB, BLK, F), dtype=np.float32)
    aux_slot = {}                     # (n, b) -> slot k
    tt = np.arange(BLK, dtype=np.int64)
    for n_i, b_i in zip(*np.nonzero(partial)):
        k = sum(1 for (nn2, _) in aux_slot if nn2 == n_i)
        assert k < AUXB, "more than AUXB partial blocks in one row"
        aux_slot[(n_i, b_i)] = k
        t = t0[b_i] + tt
        srcv = np.clip(src0[n_i] + t, 0, T - 1)
        validv = (t >= lo[n_i]) & (t < hi[n_i])
        aux[n_i, k] = np.where(validv[:, None], x[n_i, srcv, :], np.float32(0))

    in_maps = []
    for c in range(NCORES):
        rows = np.arange(c, N, NCORES)
        src_host = np.empty((SRC_ALLOC, F), dtype=np.float32)
        src_host[:N_LOC * T] = x[rows].reshape(-1, F)
        src_host[N_LOC * T:SRC_ROWS] = aux[rows].reshape(-1, F)
        src_host[SRC_ROWS:] = 0.0

        idx_host = np.full((NBLK, N_LOC), SENTINEL, dtype=np.int32)
        for n_loc, n_i in enumerate(rows):
            fb = full[n_i]
            idx_host[fb, n_loc] = (n_loc * T + src0[n_i] + t0[fb]).astype(np.int32)
            for b_i in np.nonzero(partial[n_i])[0]:
                k = aux_slot[(n_i, b_i)]
                idx_host[b_i, n_loc] = N_LOC * T + (n_loc * AUXB + k) * BLK
        in_maps.append({"src": src_host, "idx": idx_host})
    return in_maps, chunk_lens


def kernel(x, slices, lens):
    x = np.ascontiguousarray(np.asarray(x, dtype=np.float32))
    slices_in = np.asarray(slices)
    lens_in = np.asarray(lens)
    in_maps, chunk_lens = _prepare_in_maps(x, slices_in, lens_in)

    results = _get_runner(reps=1)(in_maps)

    chunks = np.empty((N, T, F), dtype=np.float32)
    for c in range(NCORES):
        chunks[c::NCORES] = results[c]["out"].reshape(N_LOC, T, F)
    return chunks, chunk_lens.astype(slices_in.dtype)


# revision 13
# speedup vs baseline: 16.4466x; 16.4466x over previous
"""ChunkBySlices Trainium2 kernel.

For each batch row n: out[n, t] = x[n, start_n + t] for t < slice_len_n, else 0
(start/end from `slices`, clamped by `lens`). Pure data movement, so the device
kernel is DMA-only:

  - N=64 rows sharded across 8 cores (strided: core c takes rows c, c+8, ...).
  - Host precomputes, per 16-row output block, a gather index into a per-core
    source buffer (the core's x shard + small host-built "aux" blocks covering
    the valid/invalid boundary + a zero guard block). Invalid blocks get an
    out-of-bounds sentinel.
  - Device: memset SBUF tile -> indirect-DMA gather (128 x 16KB descriptors,
    OOB indices skipped via bounds_check) -> contiguous 2MB store per row.

HBM traffic per core ~= 16MB write + (valid bytes) read, which is the minimum
for this op.
"""

import numpy as np

N, T, F = 64, 2048, 256
NCORES = 8
N_LOC = N // NCORES          # 8 batch rows per core
BLK = 16                     # t-rows per gather descriptor (16KB)
NBLK = T // BLK              # 128 blocks per batch row = SBUF partitions
AUXB = 2                     # aux (boundary) blocks per batch row
SRC_ROWS = N_LOC * T + N_LOC * AUXB * BLK   # 16384 + 256 = 16640
SRC_ALLOC = SRC_ROWS + BLK                  # + zero block
SENTINEL = SRC_ROWS                         # invalid blocks gather the zero block

_CACHE = {}


BUFS = 8          # SBUF slots (16KB/partition each; 8 -> 128KB of 192KB)
BOUNDS = SRC_ROWS - BLK


def _build_bass(reps=1, variant="plain"):
    """Raw-bass pipelined kernel.

    variant="plain":  every block (incl. invalid) is gathered; invalid blocks
                      read the zero block appended to src. No compute engines.
    variant="memset": DVE pre-zeros each tile; the gather skips invalid
                      (sentinel) indices via bounds_check, saving the HBM
                      reads of zeros for the invalid tail.
    """
    import concourse.bass as bass
    import concourse.mybir as mybir

    nc = bass.Bass()
    src = nc.dram_tensor("src", [SRC_ALLOC, F], mybir.dt.float32,
                         kind="ExternalInput")
    idx = nc.dram_tensor("idx", [NBLK, N_LOC], mybir.dt.int32,
                         kind="ExternalInput")
    out = nc.dram_tensor("out", [N_LOC * NBLK, BLK * F], mybir.dt.float32,
                         kind="ExternalOutput")

    G = reps * N_LOC
    use_memset = variant == "memset"
    W = BLK * F

    from contextlib import ExitStack
    with ExitStack() as es:
        big = es.enter_context(
            nc.sbuf_tensor([NBLK, BUFS * W], mybir.dt.float32))
        idxsb = es.enter_context(
            nc.sbuf_tensor([NBLK, N_LOC], mybir.dt.int32))
        # One semaphore lane per pipeline slot; at most one outstanding DMA
        # per lane, and every inc is preceded by an engine-local wait on the
        # previous boundary (sem increments of concurrent DMAs are unordered,
        # so a shared counter cannot sequence a pipeline).
        sw = [es.enter_context(nc.semaphore(f"sw{j}")) for j in range(BUFS)]
        hw = [es.enter_context(nc.semaphore(f"hw{j}")) for j in range(BUFS)]
        hx = es.enter_context(nc.semaphore("hx"))   # idx load
        ve = es.enter_context(nc.semaphore("ve"))   # memset completions (+1)
        block = es.enter_context(nc.Block())

        def slot(g):
            s = (g % BUFS) * W
            return big[:, s:s + W]

        @block.sync
        def _(sync):
            sync.dma_start(out=idxsb[:], in_=idx[:]).then_inc(hx, 16)
            for g in range(G):
                j, k = g % BUFS, g // BUFS
                sync.wait_ge(sw[j], 16 * (k + 1))     # gather g complete
                if g >= BUFS:
                    sync.wait_ge(hw[j], 16 * k)       # lane free
                n = g % N_LOC
                sync.dma_start(out=out[n * NBLK:(n + 1) * NBLK, :],
                               in_=slot(g)).then_inc(hw[j], 16)
            for j in range(BUFS):
                sync.wait_ge(hw[j], 16 * (G // BUFS))

        if use_memset:
            @block.vector
            def _(vector):
                for g in range(G):
                    j, k = g % BUFS, g // BUFS
                    if g >= BUFS:
                        # slot reused: wait for store g-BUFS to complete
                        vector.wait_ge(hw[j], 16 * k)
                    vector.memset(slot(g), 0.0).then_inc(ve, 1)

        @block.gpsimd
        def _(gpsimd):
            gpsimd.wait_ge(hx, 16)          # idx loaded
            for g in range(G):
                j, k = g % BUFS, g // BUFS
                if use_memset:
                    gpsimd.wait_ge(ve, g + 1)
                    if g >= BUFS:
                        gpsimd.wait_ge(sw[j], 16 * k)   # lane free
                elif g >= BUFS:
                    gpsimd.wait_ge(hw[j], 16 * k)   # store g-BUFS complete
                    gpsimd.wait_ge(sw[j], 16 * k)   # lane free
                n = g % N_LOC
                kw = {}
                if use_memset:
                    kw = dict(bounds_check=BOUNDS, oob_is_err=False)
                gpsimd.indirect_dma_start(
                    out=slot(g),
                    out_offset=None,
                    in_=src[:],
                    in_offset=bass.IndirectOffsetOnAxis(
                        ap=idxsb[:, n:n + 1], axis=0),
                    **kw,
                ).then_inc(sw[j], 16)

    return nc


class _Runner:
    """Compile-once PJRT executor for a Bass module on NCORES cores.

    Mirrors concourse.bass2jax.run_bass_via_pjrt's multi-core path, but keeps
    the jitted executable so repeated kernel() calls don't retrace/recompile.
    """

    def __init__(self, nc):
        import jax
        import concourse.mybir as mybir
        from concourse import bass2jax
        from jax.experimental.shard_map import shard_map
        from jax.sharding import Mesh, PartitionSpec

        bass2jax.install_neuronx_cc_hook()
        assert nc.dbg_addr is None
        partition_name = (nc.partition_id_tensor.name
                          if nc.partition_id_tensor else None)

        in_names, out_names, out_avals = [], [], []
        for alloc in nc.m.functions[0].allocations:
            if not isinstance(alloc, mybir.MemoryLocationSet):
                continue
            name = alloc.memorylocations[0].name
            if alloc.kind == "ExternalInput":
                if name != partition_name:
                    in_names.append(name)
            elif alloc.kind == "ExternalOutput":
                out_names.append(name)
                out_avals.append(jax.core.ShapedArray(
                    tuple(alloc.tensor_shape), mybir.dt.np(alloc.dtype)))
        self.in_names = list(in_names)
        self.out_names = out_names
        self.out_avals = out_avals
        n_params = len(in_names)
        all_names = in_names + out_names
        if partition_name is not None:
            all_names = all_names + [partition_name]
        donate = tuple(range(n_params, n_params + len(out_names)))

        def _body(*args):
            operands = list(args)
            if partition_name is not None:
                operands.append(bass2jax.partition_id_tensor())
            return tuple(bass2jax._bass_exec_p.bind(
                *operands,
                out_avals=tuple(out_avals),
                in_names=tuple(all_names),
                out_names=tuple(out_names),
                lowering_input_output_aliases=(),
                sim_require_finite=True,
                sim_require_nnan=True,
                nc=nc,
            ))

        devices = jax.devices()[:NCORES]
        assert len(devices) == NCORES, f"need {NCORES} cores, have {devices}"
        mesh = Mesh(np.asarray(devices), ("core",))
        self.mesh = mesh
        nin = n_params + len(out_names)
        self.sharded = jax.jit(
            shard_map(_body, mesh=mesh,
                      in_specs=(PartitionSpec("core"),) * nin,
                      out_specs=(PartitionSpec("core"),) * len(out_names),
                      check_rep=False),
            donate_argnums=donate, keep_unused=True)

    def device_inputs(self, in_maps):
        """device_put the concatenated inputs once (reusable, not donated)."""
        import jax
        from jax.sharding import NamedSharding, PartitionSpec
        sh = NamedSharding(self.mesh, PartitionSpec("core"))
        concat_in = [np.concatenate([m[name] for m in in_maps], axis=0)
                     for name in self.in_names]
        return [jax.device_put(a, sh) for a in concat_in]

    def fresh_zeros(self):
        """Donated output buffers — consumed by each call, so made fresh."""
        import jax
        from jax.sharding import NamedSharding, PartitionSpec
        sh = NamedSharding(self.mesh, PartitionSpec("core"))
        return [jax.device_put(
            np.zeros((NCORES * a.shape[0], *a.shape[1:]), a.dtype), sh)
            for a in self.out_avals]

    def run_raw(self, in_dev, zeros_dev):
        return self.sharded(*in_dev, *zeros_dev)

    def __call__(self, in_maps):
        concat_in = [np.concatenate([m[name] for m in in_maps], axis=0)
                     for name in self.in_names]
        zeros = [np.zeros((NCORES * a.shape[0], *a.shape[1:]), a.dtype)
                 for a in self.out_avals]
        out_arrs = self.sharded(*concat_in, *zeros)
        return [
            {name: np.asarray(out_arrs[i]).reshape(
                NCORES, *self.out_avals[i].shape)[c]
             for i, name in enumerate(self.out_names)}
            for c in range(NCORES)
        ]


VARIANT = "plain"


def _get_runner(reps=1, variant=None):
    variant = VARIANT if variant is None else variant
    key = ("runner", reps, variant)
    if key not in _CACHE:
        _CACHE[key] = _Runner(_build_bass(reps, variant))
    return _CACHE[key]


def _prepare_in_maps(x, slices_in, lens_in):
    """Host-side index/aux computation. Returns (in_maps, chunk_lens)."""

    start = slices_in[:, 0].astype(np.int64)
    end = slices_in[:, 1].astype(np.int64)
    lens64 = lens_in.astype(np.int64)

    chunk_lens = np.maximum(end - start, 0)
    empty = chunk_lens == 0
    left_pad = np.where(empty, 0, np.maximum(-start, 0))
    start_ = np.maximum(start, 0)
    end_ = np.minimum(end, lens64)
    slice_lens = np.maximum(end_ - start_, 0)
    lo = left_pad                     # valid t interval [lo, hi)
    hi = left_pad + slice_lens
    src0 = start_ - left_pad          # src row for t=0

    t0 = np.arange(NBLK, dtype=np.int64) * BLK          # (NBLK,)
    t1 = t0 + BLK
    full = (t0[None, :] >= lo[:, None]) & (t1[None, :] <= hi[:, None])
    none = (t1[None, :] <= lo[:, None]) | (t0[None, :] >= hi[:, None])
    partial = ~(full | none)                            # (N, NBLK)

    # Host-built boundary blocks (exact reference math on <=2 blocks per row).
    aux = np.zeros((N, AUXB, BLK, F), dtype=np.float32)
    aux_slot = {}                     # (n, b) -> slot k
    tt = np.arange(BLK, dtype=np.int64)
    for n_i, b_i in zip(*np.nonzero(partial)):
        k = sum(1 for (nn2, _) in aux_slot if nn2 == n_i)
        assert k < AUXB, "more than AUXB partial blocks in one row"
        aux_slot[(n_i, b_i)] = k
        t = t0[b_i] + tt
        srcv = np.clip(src0[n_i] + t, 0, T - 1)
        validv = (t >= lo[n_i]) & (t < hi[n_i])
        aux[n_i, k] = np.where(validv[:, None], x[n_i, srcv, :], np.float32(0))

    in_maps = []
    for c in range(NCORES):
        rows = np.arange(c, N, NCORES)
        src_host = np.empty((SRC_ALLOC, F), dtype=np.float32)
        src_host[:N_LOC * T] = x[rows].reshape(-1, F)
        src_host[N_LOC * T:SRC_ROWS] = aux[rows].reshape(-1, F)
        src_host[SRC_ROWS:] = 0.0

        idx_host = np.full((NBLK, N_LOC), SENTINEL, dtype=np.int32)
        for n_loc, n_i in enumerate(rows):
            fb = full[n_i]
            idx_host[fb, n_loc] = (n_loc * T + src0[n_i] + t0[fb]).astype(np.int32)
            for b_i in np.nonzero(partial[n_i])[0]:
                k = aux_slot[(n_i, b_i)]
                idx_host[b_i, n_loc] = N_LOC * T + (n_loc * AUXB + k) * BLK
        in_maps.append({"src": src_host, "idx": idx_host})
    return in_maps, chunk_lens


def kernel(x, slices, lens):
    x = np.ascontiguousarray(np.asarray(x, dtype=np.float32))
    slices_in = np.asarray(slices)
    lens_in = np.asarray(lens)
    in_maps, chunk_lens = _prepare_in_maps(x, slices_in, lens_in)

    results = _get_runner(reps=1)(in_maps)

    chunks = np.empty((N, T, F), dtype=np.float32)
    for c in range(NCORES):
        chunks[c::NCORES] = results[c]["out"].reshape(N_LOC, T, F)
    return chunks, chunk_lens.astype(slices_in.dtype)


# revision 15
# speedup vs baseline: 41.2389x; 2.5074x over previous
"""ChunkBySlices Trainium2 kernel.

For each batch row n: out[n, t] = x[n, start_n + t] for t < slice_len_n, else 0
(start/end from `slices`, clamped by `lens`). Pure data movement, so the device
kernel is DMA-only:

  - N=64 rows sharded across 8 cores (strided: core c takes rows c, c+8, ...).
  - Host precomputes, per 16-row output block, a gather index into a per-core
    source buffer (the core's x shard + small host-built "aux" blocks covering
    the valid/invalid boundary + a zero guard block). Invalid blocks get an
    out-of-bounds sentinel.
  - Device: memset SBUF tile -> indirect-DMA gather (128 x 16KB descriptors,
    OOB indices skipped via bounds_check) -> contiguous 2MB store per row.

HBM traffic per core ~= 16MB write + (valid bytes) read, which is the minimum
for this op.
"""

import numpy as np

N, T, F = 64, 2048, 256
NCORES = 8
N_LOC = N // NCORES          # 8 batch rows per core
BLK = 16                     # t-rows per gather descriptor (16KB)
NBLK = T // BLK              # 128 blocks per batch row = SBUF partitions
AUXB = 2                     # aux (boundary) blocks per batch row
SRC_ROWS = N_LOC * T + N_LOC * AUXB * BLK   # 16384 + 256 = 16640
SRC_ALLOC = SRC_ROWS + BLK                  # + zero block
SENTINEL = SRC_ROWS                         # invalid blocks gather the zero block

_CACHE = {}


BUFS = 8          # SBUF slots (16KB/partition each; 8 -> 128KB of 192KB)
BOUNDS = SRC_ROWS - BLK


def _build_bass(reps=1, variant="plain"):
    """Raw-bass pipelined kernel.

    variant="plain":  every block (incl. invalid) is gathered; invalid blocks
                      read the zero block appended to src. No compute engines.
    variant="memset": DVE pre-zeros each tile; the gather skips invalid
                      (sentinel) indices via bounds_check, saving the HBM
                      reads of zeros for the invalid tail.
    """
    import concourse.bass as bass
    import concourse.mybir as mybir

    nc = bass.Bass()
    src = nc.dram_tensor("src", [SRC_ALLOC, F], mybir.dt.float32,
                         kind="ExternalInput")
    idx = nc.dram_tensor("idx", [NBLK, N_LOC], mybir.dt.int32,
                         kind="ExternalInput")
    out = nc.dram_tensor("out", [N_LOC * NBLK, BLK * F], mybir.dt.float32,
                         kind="ExternalOutput")

    G = reps * N_LOC
    use_memset = variant == "memset"
    W = BLK * F

    from contextlib import ExitStack
    with ExitStack() as es:
        big = es.enter_context(
            nc.sbuf_tensor([NBLK, BUFS * W], mybir.dt.float32))
        idxsb = es.enter_context(
            nc.sbuf_tensor([NBLK, N_LOC], mybir.dt.int32))
        # One semaphore lane per pipeline slot; at most one outstanding DMA
        # per lane, and every inc is preceded by an engine-local wait on the
        # previous boundary (sem increments of concurrent DMAs are unordered,
        # so a shared counter cannot sequence a pipeline).
        sw = [es.enter_context(nc.semaphore(f"sw{j}")) for j in range(BUFS)]
        hw = [es.enter_context(nc.semaphore(f"hw{j}")) for j in range(BUFS)]
        hx = es.enter_context(nc.semaphore("hx"))   # idx load
        ve = es.enter_context(nc.semaphore("ve"))   # memset completions (+1)
        block = es.enter_context(nc.Block())

        def slot(g):
            s = (g % BUFS) * W
            return big[:, s:s + W]

        @block.sync
        def _(sync):
            sync.dma_start(out=idxsb[:], in_=idx[:]).then_inc(hx, 16)
            for g in range(G):
                j, k = g % BUFS, g // BUFS
                sync.wait_ge(sw[j], 16 * (k + 1))     # gather g complete
                if g >= BUFS:
                    sync.wait_ge(hw[j], 16 * k)       # lane free
                n = g % N_LOC
                sync.dma_start(out=out[n * NBLK:(n + 1) * NBLK, :],
                               in_=slot(g)).then_inc(hw[j], 16)
            for j in range(BUFS):
                sync.wait_ge(hw[j], 16 * (G // BUFS))

        if use_memset:
            @block.vector
            def _(vector):
                for g in range(G):
                    j, k = g % BUFS, g // BUFS
                    if g >= BUFS:
                        # slot reused: wait for store g-BUFS to complete
                        vector.wait_ge(hw[j], 16 * k)
                    vector.memset(slot(g), 0.0).then_inc(ve, 1)

        @block.gpsimd
        def _(gpsimd):
            breg = gpsimd.to_reg(BOUNDS) if use_memset else None
            gpsimd.wait_ge(hx, 16)          # idx loaded
            for g in range(G):
                j, k = g % BUFS, g // BUFS
                if use_memset:
                    gpsimd.wait_ge(ve, g + 1)
                    if g >= BUFS:
                        gpsimd.wait_ge(sw[j], 16 * k)   # lane free
                elif g >= BUFS:
                    gpsimd.wait_ge(hw[j], 16 * k)   # store g-BUFS complete
                    gpsimd.wait_ge(sw[j], 16 * k)   # lane free
                n = g % N_LOC
                kw = {}
                if use_memset:
                    kw = dict(bounds_check=breg, oob_is_err=False)
                gpsimd.indirect_dma_start(
                    out=slot(g),
                    out_offset=None,
                    in_=src[:],
                    in_offset=bass.IndirectOffsetOnAxis(
                        ap=idxsb[:, n:n + 1], axis=0),
                    **kw,
                ).then_inc(sw[j], 16)

    return nc


class _Runner:
    """Compile-once PJRT executor for a Bass module on NCORES cores.

    Mirrors concourse.bass2jax.run_bass_via_pjrt's multi-core path, but keeps
    the jitted executable so repeated kernel() calls don't retrace/recompile.
    """

    def __init__(self, nc):
        import jax
        import concourse.mybir as mybir
        from concourse import bass2jax
        from jax.experimental.shard_map import shard_map
        from jax.sharding import Mesh, PartitionSpec

        bass2jax.install_neuronx_cc_hook()
        assert nc.dbg_addr is None
        partition_name = (nc.partition_id_tensor.name
                          if nc.partition_id_tensor else None)

        in_names, out_names, out_avals = [], [], []
        for alloc in nc.m.functions[0].allocations:
            if not isinstance(alloc, mybir.MemoryLocationSet):
                continue
            name = alloc.memorylocations[0].name
            if alloc.kind == "ExternalInput":
                if name != partition_name:
                    in_names.append(name)
            elif alloc.kind == "ExternalOutput":
                out_names.append(name)
                out_avals.append(jax.core.ShapedArray(
                    tuple(alloc.tensor_shape), mybir.dt.np(alloc.dtype)))
        self.in_names = list(in_names)
        self.out_names = out_names
        self.out_avals = out_avals
        n_params = len(in_names)
        all_names = in_names + out_names
        if partition_name is not None:
            all_names = all_names + [partition_name]
        donate = tuple(range(n_params, n_params + len(out_names)))

        def _body(*args):
            operands = list(args)
            if partition_name is not None:
                operands.append(bass2jax.partition_id_tensor())
            return tuple(bass2jax._bass_exec_p.bind(
                *operands,
                out_avals=tuple(out_avals),
                in_names=tuple(all_names),
                out_names=tuple(out_names),
                lowering_input_output_aliases=(),
                sim_require_finite=True,
                sim_require_nnan=True,
                nc=nc,
            ))

        devices = jax.devices()[:NCORES]
        assert len(devices) == NCORES, f"need {NCORES} cores, have {devices}"
        mesh = Mesh(np.asarray(devices), ("core",))
        self.mesh = mesh
        nin = n_params + len(out_names)
        self.sharded = jax.jit(
            shard_map(_body, mesh=mesh,
                      in_specs=(PartitionSpec("core"),) * nin,
                      out_specs=(PartitionSpec("core"),) * len(out_names),
                      check_rep=False),
            donate_argnums=donate, keep_unused=True)

    def device_inputs(self, in_maps):
        """device_put the concatenated inputs once (reusable, not donated)."""
        import jax
        from jax.sharding import NamedSharding, PartitionSpec
        sh = NamedSharding(self.mesh, PartitionSpec("core"))
        concat_in = [np.concatenate([m[name] for m in in_maps], axis=0)
                     for name in self.in_names]
        return [jax.device_put(a, sh) for a in concat_in]

    def fresh_zeros(self):
        """Donated output buffers — consumed by each call, so made fresh."""
        import jax
        from jax.sharding import NamedSharding, PartitionSpec
        sh = NamedSharding(self.mesh, PartitionSpec("core"))
        return [jax.device_put(
            np.zeros((NCORES * a.shape[0], *a.shape[1:]), a.dtype), sh)
            for a in self.out_avals]

    def run_raw(self, in_dev, zeros_dev):
        return self.sharded(*in_dev, *zeros_dev)

    def __call__(self, in_maps):
        concat_in = [np.concatenate([m[name] for m in in_maps], axis=0)
                     for name in self.in_names]
        zeros = [np.zeros((NCORES * a.shape[0], *a.shape[1:]), a.dtype)
                 for a in self.out_avals]
        out_arrs = self.sharded(*concat_in, *zeros)
        return [
            {name: np.asarray(out_arrs[i]).reshape(
                NCORES, *self.out_avals[i].shape)[c]
             for i, name in enumerate(self.out_names)}
            for c in range(NCORES)
        ]


VARIANT = "plain"


def _get_runner(reps=1, variant=None):
    variant = VARIANT if variant is None else variant
    key = ("runner", reps, variant)
    if key not in _CACHE:
        _CACHE[key] = _Runner(_build_bass(reps, variant))
    return _CACHE[key]


def _prepare_in_maps(x, slices_in, lens_in):
    """Host-side index/aux computation. Returns (in_maps, chunk_lens)."""

    start = slices_in[:, 0].astype(np.int64)
    end = slices_in[:, 1].astype(np.int64)
    lens64 = lens_in.astype(np.int64)

    chunk_lens = np.maximum(end - start, 0)
    empty = chunk_lens == 0
    left_pad = np.where(empty, 0, np.maximum(-start, 0))
    start_ = np.maximum(start, 0)
    end_ = np.minimum(end, lens64)
    slice_lens = np.maximum(end_ - start_, 0)
    lo = left_pad                     # valid t interval [lo, hi)
    hi = left_pad + slice_lens
    src0 = start_ - left_pad          # src row for t=0

    t0 = np.arange(NBLK, dtype=np.int64) * BLK          # (NBLK,)
    t1 = t0 + BLK
    full = (t0[None, :] >= lo[:, None]) & (t1[None, :] <= hi[:, None])
    none = (t1[None, :] <= lo[:, None]) | (t0[None, :] >= hi[:, None])
    partial = ~(full | none)                            # (N, NBLK)

    # Host-built boundary blocks (exact reference math on <=2 blocks per row).
    aux = np.zeros((N, AUXB, BLK, F), dtype=np.float32)
    aux_slot = {}                     # (n, b) -> slot k
    tt = np.arange(BLK, dtype=np.int64)
    for n_i, b_i in zip(*np.nonzero(partial)):
        k = sum(1 for (nn2, _) in aux_slot if nn2 == n_i)
        assert k < AUXB, "more than AUXB partial blocks in one row"
        aux_slot[(n_i, b_i)] = k
        t = t0[b_i] + tt
        srcv = np.clip(src0[n_i] + t, 0, T - 1)
        validv = (t >= lo[n_i]) & (t < hi[n_i])
        aux[n_i, k] = np.where(validv[:, None], x[n_i, srcv, :], np.float32(0))

    in_maps = []
    for c in range(NCORES):
        rows = np.arange(c, N, NCORES)
        src_host = np.empty((SRC_ALLOC, F), dtype=np.float32)
        src_host[:N_LOC * T] = x[rows].reshape(-1, F)
        src_host[N_LOC * T:SRC_ROWS] = aux[rows].reshape(-1, F)
        src_host[SRC_ROWS:] = 0.0

        idx_host = np.full((NBLK, N_LOC), SENTINEL, dtype=np.int32)
        for n_loc, n_i in enumerate(rows):
            fb = full[n_i]
            idx_host[fb, n_loc] = (n_loc * T + src0[n_i] + t0[fb]).astype(np.int32)
            for b_i in np.nonzero(partial[n_i])[0]:
                k = aux_slot[(n_i, b_i)]
                idx_host[b_i, n_loc] = N_LOC * T + (n_loc * AUXB + k) * BLK
        in_maps.append({"src": src_host, "idx": idx_host})
    return in_maps, chunk_lens


def kernel(x, slices, lens):
    x = np.ascontiguousarray(np.asarray(x, dtype=np.float32))
    slices_in = np.asarray(slices)
    lens_in = np.asarray(lens)
    in_maps, chunk_lens = _prepare_in_maps(x, slices_in, lens_in)

    results = _get_runner(reps=1)(in_maps)

    chunks = np.empty((N, T, F), dtype=np.float32)
    for c in range(NCORES):
        chunks[c::NCORES] = results[c]["out"].reshape(N_LOC, T, F)
    return chunks, chunk_lens.astype(slices_in.dtype)


# revision 16
# speedup vs baseline: 110.9094x; 2.6894x over previous
"""ChunkBySlices Trainium2 kernel.

For each batch row n: out[n, t] = x[n, start_n + t] for t < slice_len_n, else 0
(start/end from `slices`, clamped by `lens`). Pure data movement, so the device
kernel is DMA-only:

  - N=64 rows sharded across 8 cores (strided: core c takes rows c, c+8, ...).
  - Host precomputes, per 16-row output block, a gather index into a per-core
    source buffer (the core's x shard + small host-built "aux" blocks covering
    the valid/invalid boundary + a zero guard block). Invalid blocks get an
    out-of-bounds sentinel.
  - Device: memset SBUF tile -> indirect-DMA gather (128 x 16KB descriptors,
    OOB indices skipped via bounds_check) -> contiguous 2MB store per row.

HBM traffic per core ~= 16MB write + (valid bytes) read, which is the minimum
for this op.
"""

import numpy as np

N, T, F = 64, 2048, 256
NCORES = 8
N_LOC = N // NCORES          # 8 batch rows per core
BLK = 16                     # t-rows per gather descriptor (16KB)
NBLK = T // BLK              # 128 blocks per batch row = SBUF partitions
AUXB = 2                     # aux (boundary) blocks per batch row
SRC_ROWS = N_LOC * T + N_LOC * AUXB * BLK   # 16384 + 256 = 16640
SRC_ALLOC = SRC_ROWS + BLK                  # + zero block
SENTINEL = SRC_ROWS                         # invalid blocks gather the zero block

_CACHE = {}


BUFS = 8          # SBUF slots (16KB/partition each; 8 -> 128KB of 192KB)
BOUNDS = SRC_ROWS - BLK


def _build_bass(reps=1, variant="plain"):
    """Raw-bass pipelined kernel.

    variant="plain":  every block (incl. invalid) is gathered; invalid blocks
                      read the zero block appended to src. No compute engines.
    variant="memset": DVE pre-zeros each tile; the gather skips invalid
                      (sentinel) indices via bounds_check, saving the HBM
                      reads of zeros for the invalid tail.
    """
    import concourse.bass as bass
    import concourse.mybir as mybir

    nc = bass.Bass()
    src = nc.dram_tensor("src", [SRC_ALLOC, F], mybir.dt.float32,
                         kind="ExternalInput")
    idx = nc.dram_tensor("idx", [NBLK, N_LOC], mybir.dt.int32,
                         kind="ExternalInput")
    out = nc.dram_tensor("out", [N_LOC * NBLK, BLK * F], mybir.dt.float32,
                         kind="ExternalOutput")

    G = reps * N_LOC
    use_memset = variant == "memset"
    W = BLK * F

    from contextlib import ExitStack
    with ExitStack() as es:
        big = es.enter_context(
            nc.sbuf_tensor([NBLK, BUFS * W], mybir.dt.float32))
        idxsb = es.enter_context(
            nc.sbuf_tensor([NBLK, N_LOC], mybir.dt.int32))
        # One semaphore lane per pipeline slot; at most one outstanding DMA
        # per lane, and every inc is preceded by an engine-local wait on the
        # previous boundary (sem increments of concurrent DMAs are unordered,
        # so a shared counter cannot sequence a pipeline).
        sw = [es.enter_context(nc.semaphore(f"sw{j}")) for j in range(BUFS)]
        hw = [es.enter_context(nc.semaphore(f"hw{j}")) for j in range(BUFS)]
        hx = es.enter_context(nc.semaphore("hx"))   # idx load
        ve = es.enter_context(nc.semaphore("ve"))   # memset completions (+1)
        block = es.enter_context(nc.Block())

        def slot(g):
            s = (g % BUFS) * W
            return big[:, s:s + W]

        @block.sync
        def _(sync):
            sync.dma_start(out=idxsb[:], in_=idx[:]).then_inc(hx, 16)
            for g in range(G):
                j, k = g % BUFS, g // BUFS
                sync.wait_ge(sw[j], 16 * (k + 1))     # gather g complete
                if g >= BUFS:
                    sync.wait_ge(hw[j], 16 * k)       # lane free
                n = g % N_LOC
                sync.dma_start(out=out[n * NBLK:(n + 1) * NBLK, :],
                               in_=slot(g)).then_inc(hw[j], 16)
            for j in range(BUFS):
                sync.wait_ge(hw[j], 16 * (G // BUFS))

        if use_memset:
            @block.vector
            def _(vector):
                for g in range(G):
                    j, k = g % BUFS, g // BUFS
                    if g >= BUFS:
                        # slot reused: wait for store g-BUFS to complete
                        vector.wait_ge(hw[j], 16 * k)
                    vector.memset(slot(g), 0.0).then_inc(ve, 1)

        @block.gpsimd
        def _(gpsimd):
            breg = gpsimd.to_reg(BOUNDS) if use_memset else None
            gpsimd.wait_ge(hx, 16)          # idx loaded
            for g in range(G):
                j, k = g % BUFS, g // BUFS
                if use_memset:
                    gpsimd.wait_ge(ve, g + 1)
                    if g >= BUFS:
                        gpsimd.wait_ge(sw[j], 16 * k)   # lane free
                elif g >= BUFS:
                    gpsimd.wait_ge(hw[j], 16 * k)   # store g-BUFS complete
                    gpsimd.wait_ge(sw[j], 16 * k)   # lane free
                n = g % N_LOC
                kw = {}
                if use_memset:
                    kw = dict(bounds_check=breg, oob_is_err=False)
                gpsimd.indirect_dma_start(
                    out=slot(g),
                    out_offset=None,
                    in_=src[:],
                    in_offset=bass.IndirectOffsetOnAxis(
                        ap=idxsb[:, n:n + 1], axis=0),
                    **kw,
                ).then_inc(sw[j], 16)

    return nc


class _Runner:
    """Compile-once PJRT executor for a Bass module on NCORES cores.

    Mirrors concourse.bass2jax.run_bass_via_pjrt's multi-core path, but keeps
    the jitted executable so repeated kernel() calls don't retrace/recompile.
    """

    def __init__(self, nc):
        import jax
        import concourse.mybir as mybir
        from concourse import bass2jax
        from jax.experimental.shard_map import shard_map
        from jax.sharding import Mesh, PartitionSpec

        bass2jax.install_neuronx_cc_hook()
        assert nc.dbg_addr is None
        partition_name = (nc.partition_id_tensor.name
                          if nc.partition_id_tensor else None)

        in_names, out_names, out_avals = [], [], []
        for alloc in nc.m.functions[0].allocations:
            if not isinstance(alloc, mybir.MemoryLocationSet):
                continue
            name = alloc.memorylocations[0].name
            if alloc.kind == "ExternalInput":
                if name != partition_name:
                    in_names.append(name)
            elif alloc.kind == "ExternalOutput":
                out_names.append(name)
                out_avals.append(jax.core.ShapedArray(
                    tuple(alloc.tensor_shape), mybir.dt.np(alloc.dtype)))
        self.in_names = list(in_names)
        self.out_names = out_names
        self.out_avals = out_avals
        n_params = len(in_names)
        all_names = in_names + out_names
        if partition_name is not None:
            all_names = all_names + [partition_name]
        donate = tuple(range(n_params, n_params + len(out_names)))

        def _body(*args):
            operands = list(args)
            if partition_name is not None:
                operands.append(bass2jax.partition_id_tensor())
            return tuple(bass2jax._bass_exec_p.bind(
                *operands,
                out_avals=tuple(out_avals),
                in_names=tuple(all_names),
                out_names=tuple(out_names),
                lowering_input_output_aliases=(),
                sim_require_finite=True,
                sim_require_nnan=True,
                nc=nc,
            ))

        devices = jax.devices()[:NCORES]
        assert len(devices) == NCORES, f"need {NCORES} cores, have {devices}"
        mesh = Mesh(np.asarray(devices), ("core",))
        self.mesh = mesh
        nin = n_params + len(out_names)
        self.sharded = jax.jit(
            shard_map(_body, mesh=mesh,
                      in_specs=(PartitionSpec("core"),) * nin,
                      out_specs=(PartitionSpec("core"),) * len(out_names),
                      check_rep=False),
            donate_argnums=donate, keep_unused=True)

    def device_inputs(self, in_maps):
        """device_put the concatenated inputs once (reusable, not donated)."""
        import jax
        from jax.sharding import NamedSharding, PartitionSpec
        sh = NamedSharding(self.mesh, PartitionSpec("core"))
        concat_in = [np.concatenate([m[name] for m in in_maps], axis=0)
                     for name in self.in_names]
        return [jax.device_put(a, sh) for a in concat_in]

    def fresh_zeros(self):
        """Donated output buffers — consumed by each call, so made fresh."""
        import jax
        from jax.sharding import NamedSharding, PartitionSpec
        sh = NamedSharding(self.mesh, PartitionSpec("core"))
        return [jax.device_put(
            np.zeros((NCORES * a.shape[0], *a.shape[1:]), a.dtype), sh)
            for a in self.out_avals]

    def run_raw(self, in_dev, zeros_dev):
        return self.sharded(*in_dev, *zeros_dev)

    def __call__(self, in_maps):
        concat_in = [np.concatenate([m[name] for m in in_maps], axis=0)
                     for name in self.in_names]
        zeros = [np.zeros((NCORES * a.shape[0], *a.shape[1:]), a.dtype)
                 for a in self.out_avals]
        out_arrs = self.sharded(*concat_in, *zeros)
        return [
            {name: np.asarray(out_arrs[i]).reshape(
                NCORES, *self.out_avals[i].shape)[c]
             for i, name in enumerate(self.out_names)}
            for c in range(NCORES)
        ]


VARIANT = "memset"


def _get_runner(reps=1, variant=None):
    variant = VARIANT if variant is None else variant
    key = ("runner", reps, variant)
    if key not in _CACHE:
        _CACHE[key] = _Runner(_build_bass(reps, variant))
    return _CACHE[key]


def _prepare_in_maps(x, slices_in, lens_in):
    """Host-side index/aux computation. Returns (in_maps, chunk_lens)."""

    start = slices_in[:, 0].astype(np.int64)
    end = slices_in[:, 1].astype(np.int64)
    lens64 = lens_in.astype(np.int64)

    chunk_lens = np.maximum(end - start, 0)
    empty = chunk_lens == 0
    left_pad = np.where(empty, 0, np.maximum(-start, 0))
    start_ = np.maximum(start, 0)
    end_ = np.minimum(end, lens64)
    slice_lens = np.maximum(end_ - start_, 0)
    lo = left_pad                     # valid t interval [lo, hi)
    hi = left_pad + slice_lens
    src0 = start_ - left_pad          # src row for t=0

    t0 = np.arange(NBLK, dtype=np.int64) * BLK          # (NBLK,)
    t1 = t0 + BLK
    full = (t0[None, :] >= lo[:, None]) & (t1[None, :] <= hi[:, None])
    none = (t1[None, :] <= lo[:, None]) | (t0[None, :] >= hi[:, None])
    partial = ~(full | none)                            # (N, NBLK)

    # Host-built boundary blocks (exact reference math on <=2 blocks per row).
    aux = np.zeros((N, AUXB, BLK, F), dtype=np.float32)
    aux_slot = {}                     # (n, b) -> slot k
    tt = np.arange(BLK, dtype=np.int64)
    for n_i, b_i in zip(*np.nonzero(partial)):
        k = sum(1 for (nn2, _) in aux_slot if nn2 == n_i)
        assert k < AUXB, "more than AUXB partial blocks in one row"
        aux_slot[(n_i, b_i)] = k
        t = t0[b_i] + tt
        srcv = np.clip(src0[n_i] + t, 0, T - 1)
        validv = (t >= lo[n_i]) & (t < hi[n_i])
        aux[n_i, k] = np.where(validv[:, None], x[n_i, srcv, :], np.float32(0))

    in_maps = []
    for c in range(NCORES):
        rows = np.arange(c, N, NCORES)
        src_host = np.empty((SRC_ALLOC, F), dtype=np.float32)
        src_host[:N_LOC * T] = x[rows].reshape(-1, F)
        src_host[N_LOC * T:SRC_ROWS] = aux[rows].reshape(-1, F)
        src_host[SRC_ROWS:] = 0.0

        idx_host = np.full((NBLK, N_LOC), SENTINEL, dtype=np.int32)
        for n_loc, n_i in enumerate(rows):
            fb = full[n_i]
            idx_host[fb, n_loc] = (n_loc * T + src0[n_i] + t0[fb]).astype(np.int32)
            for b_i in np.nonzero(partial[n_i])[0]:
                k = aux_slot[(n_i, b_i)]
                idx_host[b_i, n_loc] = N_LOC * T + (n_loc * AUXB + k) * BLK
        in_maps.append({"src": src_host, "idx": idx_host})
    return in_maps, chunk_lens


def kernel(x, slices, lens):
    x = np.ascontiguousarray(np.asarray(x, dtype=np.float32))
    slices_in = np.asarray(slices)
    lens_in = np.asarray(lens)
    in_maps, chunk_lens = _prepare_in_maps(x, slices_in, lens_in)

    results = _get_runner(reps=1)(in_maps)

    chunks = np.empty((N, T, F), dtype=np.float32)
    for c in range(NCORES):
        chunks[c::NCORES] = results[c]["out"].reshape(N_LOC, T, F)
    return chunks, chunk_lens.astype(slices_in.dtype)
